# revision 22
# baseline (speedup 1.0000x reference)
"""Trainium2 Bass kernel for masked-softmax attention scoring (v3).

Reference computation (B=128, T=512, K=1024, Q=1024):
    mids  = einsum("kq,bq->bk", W, query)
    s     = tanh(einsum("btk,bk->bt", key, mids) + bias)
    attn  = softmax-like: exp(s - max) * mask / sum(exp(s - max) * mask)

The max-subtraction cancels exactly in the ratio (tanh is bounded), so the
device computes  attn = exp(tanh(.)) * mask / sum_t(exp(tanh(.)) * mask).

Design (evolved from the v1 DVE kernel via trace analysis):
- Everything 16-bit on the wire: key/W/query cast to fp16 on the host
  (rel_l2 ~1.4e-3 vs the 2e-2 gate), halving the dominant HBM stream.
- Score dot-products run on the TensorEngine (not the DVE, whose fused
  mul-reduce is 1.13 us per 1024-col column): the host pre-transposes key
  to [pair, kc-pair, k-partition, kcsub, (b0 t | b1 t)] so the PE
  contracts over k on partitions.  1 MB tiles (4 matmuls each) keep the
  per-dma_start sequencer cost (~630 ns) off the critical path; a single
  sync-queue ring fans out across all 16 hardware DMA queues.
- PSUM accumulation groups must each own a full bank (interleaved groups
  at different byte offsets within one bank accumulate incorrectly):
  mids^T uses 8 banks (kc -> bank kc, qc-outer so matmuls overlap W's
  arrival); scores then reuse 4 of those banks, one per in-flight batch
  (bank = 2*(pr%2) + h, WAR distance = one full pair).
- The otherwise-idle Scalar engine extracts each batch row from PSUM row 0
  with a fused Tanh, then Exp; the DVE does the per-batch mask+rowsum
  (affine_mul_reduce), reciprocal, and scale -- all pipelined behind the
  PE stream.  gpsimd (software DGE) issues the 16 tiny row DMAs out, so
  no hardware ring ever blocks on a compute dependency.
- Mask compaction (COMPACT=True): masked-out timesteps (~20%) are never
  shipped; the host gathers kept t's per batch, the device computes only
  Tc = max kept count columns, the host scatters rows back.  The NEFF is
  compiled for the actual Tc on first call.

Sharding: data-parallel over B across 8 NeuronCores (16 batches/core).
"""

import sys

if "/opt/trn_rl_repo" not in sys.path:
    sys.path.insert(0, "/opt/trn_rl_repo")

from contextlib import ExitStack

import numpy as np

# ---- problem constants (hardcoded per spec) ----
B, T, K, Q = 128, 512, 1024, 1024
NCORES = 8
BS = B // NCORES          # 16 batches per core
P = 128                   # SBUF partitions
QC = Q // P               # 8 contraction chunks for the mids matmul
KC = K // P               # 8 contraction chunks for the scores matmul
KCP = KC // 2             # kc pairs per key tile
PR = BS // 2              # 8 batch pairs per core (2 batches per key tile)
KEY_BUFS = 10             # key tile pool depth (4 KB/partition each)
COMPACT = True           # gather kept timesteps on host, Tc = max count
JB = 64                   # j-blocks per DVE-pair batch (2*JB = 128 partitions)

_STATE: dict = {}


def _build_nc(Tc):
    import concourse.tile as tile
    from concourse import bacc, mybir

    f32 = mybir.dt.float32
    f16 = mybir.dt.float16
    nc = bacc.Bacc()

    qt_e = nc.declare_dram_parameter("qt", [P, QC, BS], f16, isOutput=False)
    wt_e = nc.declare_dram_parameter("wt", [P, KC, QC, P], f16, isOutput=False)
    keyt_e = nc.declare_dram_parameter(
        "keyt", [PR - 1, KCP, P, 2, 2 * Tc], f16, isOutput=False
    )
    maskb_e = nc.declare_dram_parameter("maskb", [BS, Tc], f32, isOutput=False)
    bias_e = nc.declare_dram_parameter("biasb", [P, 1], f32, isOutput=False)
    out_e = nc.declare_dram_parameter("out", [BS - 2, Tc], f32, isOutput=True)
    # DVE-offloaded last pair: batches 14/15 in (h, j)-partition row layout
    CFJ = -(-Tc // JB)
    keyj_e = nc.declare_dram_parameter("keyj", [P, CFJ, K], f16, isOutput=False)
    maskj_e = nc.declare_dram_parameter("maskj", [P, CFJ], f32, isOutput=False)
    grpj_e = nc.declare_dram_parameter("grpj", [P, P], f32, isOutput=False)
    idn_e = nc.declare_dram_parameter("idn", [P, P], f16, isOutput=False)
    outj_e = nc.declare_dram_parameter("outj", [P, CFJ], f32, isOutput=True)

    with tile.TileContext(nc) as tc, ExitStack() as ctx:
        const = ctx.enter_context(tc.tile_pool(name="const", bufs=1))
        kpool = ctx.enter_context(tc.tile_pool(name="key", bufs=KEY_BUFS))
        psum = ctx.enter_context(tc.tile_pool(name="psum", bufs=1, space="PSUM"))

        # 8 full psum banks: mids kc-group kc lives in pb[kc][:, :BS];
        # scores then reuse pb[0..3] row 0
        pb = [psum.tile([P, 512], f32, name=f"pb{i}") for i in range(8)]

        # ---- prologue loads ----
        # W streams kc-major: mids group kc only needs its own 256 KB
        # slice.  Slice 0 leads the sync ring (ahead of even qt) and the
        # rest ride the scalar ring, so the sync ring reaches the first
        # key tile almost immediately.
        wt_sb = const.tile([P, KC, QC, P], f16)
        nc.sync.dma_start(out=wt_sb[:, 0, :, :], in_=wt_e[:, 0, :, :])
        qt_sb = const.tile([P, QC, BS], f16)
        nc.sync.dma_start(out=qt_sb[:], in_=qt_e[:])
        for kc in range(1, KC):
            nc.scalar.dma_start(out=wt_sb[:, kc, :, :], in_=wt_e[:, kc, :, :])
        maskb_sb = const.tile([1, BS, Tc], f32)
        nc.scalar.dma_start(
            out=maskb_sb[:], in_=maskb_e[:].rearrange("(o b) t -> o b t", o=1)
        )
        bias_sb = const.tile([P, 1], f32)
        nc.scalar.dma_start(out=bias_sb[:], in_=bias_e[:])
        maskj_sb = const.tile([P, CFJ], f32)
        nc.scalar.dma_start(out=maskj_sb[:], in_=maskj_e[:])
        grpj_sb = const.tile([P, P], f32)
        nc.scalar.dma_start(out=grpj_sb[:], in_=grpj_e[:])
        idn_sb = const.tile([P, P], f16)
        nc.scalar.dma_start(out=idn_sb[:], in_=idn_e[:])

        # ---- midsT[p, kc, b] = mids[b, kc*128+p] ----
        # kc-outer groups (one open accumulation group per bank, banks 4-7
        # rotating), with each group's midsT slice copied right after its
        # stop.  Groups for kc pair X are emitted just before the score
        # matmuls that need them, so mids work fills PE gaps in the
        # DMA-paced stream instead of serializing ahead of it.
        midsT_sb = const.tile([P, KC, BS], f16)

        def mids_group(kc):
            bank = 4 + kc % 4
            for qc in range(QC):
                nc.tensor.matmul(
                    pb[bank][:, :BS],
                    lhsT=wt_sb[:, kc, qc, :],
                    rhs=qt_sb[:, qc, :],
                    start=(qc == 0),
                    stop=(qc == QC - 1),
                )
            nc.vector.tensor_copy(midsT_sb[:, kc, :], pb[bank][:, :BS])

        # per-batch partition-0 tiles for the pipelined epilogue
        NPE = BS - 2
        tanh_t = [const.tile([1, Tc], f32, name=f"tanh{b}") for b in range(NPE)]
        exp_t = [const.tile([1, Tc], f32, name=f"exp{b}") for b in range(NPE)]
        rsum_t = [const.tile([1, 1], f32, name=f"rsum{b}") for b in range(NPE)]
        rinv_t = [const.tile([1, 1], f32, name=f"rinv{b}") for b in range(NPE)]

        # DVE-pair working tiles (batches 14/15, v1-style row layout)
        kj_t = []
        for ci in range(0, CFJ, 4):
            w = min(4, CFJ - ci)
            kj = const.tile([P, w, K], f16, name=f"kj{ci}")
            kj_t.append((ci, w, kj))
        midsT_rep = const.tile([P, KC, 2, JB], f16)
        mids_bcJ = const.tile([P, K], f16)
        prodJ = const.tile([P, K], f16)
        scoresJ = const.tile([P, CFJ], f32)
        tanhJ = const.tile([P, CFJ], f32)
        expJ = const.tile([P, CFJ], f32)
        emJ = const.tile([P, CFJ], f32)
        rsumJ = const.tile([P, 1], f32)
        rinvJ = const.tile([P, 1], f32)
        attnJ = const.tile([P, CFJ], f32)

        def mids_bcj_build():
            # replicate mids columns of batches 14/15 into (h, j) layout,
            # then transpose k onto the free axis via identity matmuls
            nc.vector.tensor_copy(
                midsT_rep[:],
                midsT_sb[:, :, BS - 2 : BS]
                .unsqueeze(-1)
                .broadcast_to((P, KC, 2, JB)),
            )
            for kc in range(KC):
                bank = 6 + kc // 4
                nc.tensor.matmul(
                    pb[bank][:, (kc % 4) * P : (kc % 4 + 1) * P],
                    lhsT=midsT_rep[:, kc, :, :],
                    rhs=idn_sb[:],
                    start=True,
                    stop=True,
                )
            nc.vector.tensor_copy(mids_bcJ[:, 0:512], pb[6][:])
            nc.vector.tensor_copy(mids_bcJ[:, 512:1024], pb[7][:])

        def dve_col(c):
            nc.vector.affine_mul_reduce(
                out=prodJ[:],
                accum_out=scoresJ[:, c : c + 1],
                in0=kj_t[c // 4][2][:, c % 4, :],
                in1=mids_bcJ[:],
                scale=1.0,
                bias=0.0,
            )

        mids_group(0)
        mids_group(1)

        # ---- stream 1 MB key tiles on the sync ring; 4 matmuls per tile ----
        dve_cols = list(range(CFJ))
        for pr in range(PR - 1):
            for kcp in range(KCP):
                if pr == 0 and kcp > 0:
                    mids_group(2 * kcp)
                    mids_group(2 * kcp + 1)
                    if kcp == KCP - 1:
                        mids_bcj_build()
                if kcp == 1 and pr < len(kj_t):
                    ci, w, kj = kj_t[pr]
                    nc.sync.dma_start(out=kj[:], in_=keyj_e[:, ci : ci + w, :])
                kt = kpool.tile([P, 2, 2 * Tc], f16, tag="kt")
                nc.sync.dma_start(out=kt[:], in_=keyt_e[pr, kcp, :, :, :])
                for h in range(2):
                    b = 2 * pr + h
                    bank = 2 * (pr % 2) + h
                    for j in range(2):
                        kc = 2 * kcp + j
                        nc.tensor.matmul(
                            pb[bank][0:1, :Tc],
                            lhsT=midsT_sb[:, kc, b : b + 1],
                            rhs=kt[:, j, h * Tc : (h + 1) * Tc],
                            start=(kc == 0),
                            stop=(kc == KC - 1),
                        )
            # pipelined per-batch epilogue: scalar does fused extract+tanh
            # from PSUM then exp; vector does mask+rowsum, reciprocal,
            # scale; gpsimd (software DGE) DMAs the finished row out.
            # Buffer reuse: masked-exp overwrites tanh_t, attn overwrites
            # exp_t.
            for h in range(2):
                b = 2 * pr + h
                bank = 2 * (pr % 2) + h
                nc.scalar.activation(
                    out=tanh_t[b][:],
                    in_=pb[bank][0:1, :Tc],
                    func=mybir.ActivationFunctionType.Tanh,
                    bias=bias_sb[0:1, :],
                    scale=1.0,
                )
                nc.scalar.activation(
                    out=exp_t[b][:],
                    in_=tanh_t[b][:],
                    func=mybir.ActivationFunctionType.Exp,
                )
                nc.vector.affine_mul_reduce(
                    out=tanh_t[b][:],
                    accum_out=rsum_t[b][:],
                    in0=exp_t[b][:],
                    in1=maskb_sb[:, b, :],
                    scale=1.0,
                    bias=0.0,
                )
                nc.vector.reciprocal(out=rinv_t[b][:], in_=rsum_t[b][:])
                nc.vector.tensor_scalar_mul(exp_t[b][:], tanh_t[b][:], rinv_t[b][:])
            for h in range(2):
                b = 2 * pr + h
                nc.scalar.dma_start(out=out_e[b : b + 1, :], in_=exp_t[b][:])
            # interleave DVE-pair score columns into the vector queue
            if pr >= 1:
                n_emit = 1 if len(dve_cols) <= (PR - 1 - pr - 1) + 1 else 2
                for _ in range(n_emit):
                    if dve_cols:
                        dve_col(dve_cols.pop(0))

        # ---- DVE-pair tail: tanh/exp, mask+rowsum, block group-sum,
        # normalize, out ----
        while dve_cols:
            dve_col(dve_cols.pop(0))
        nc.scalar.activation(
            out=tanhJ[:],
            in_=scoresJ[:],
            func=mybir.ActivationFunctionType.Tanh,
            bias=bias_sb[:],
            scale=1.0,
        )
        nc.scalar.activation(
            out=expJ[:], in_=tanhJ[:], func=mybir.ActivationFunctionType.Exp
        )
        nc.vector.affine_mul_reduce(
            out=emJ[:],
            accum_out=rsumJ[:],
            in0=expJ[:],
            in1=maskj_sb[:],
            scale=1.0,
            bias=0.0,
        )
        nc.tensor.matmul(
            pb[5][:, 0:1], lhsT=grpj_sb[:], rhs=rsumJ[:], start=True, stop=True
        )
        nc.vector.reciprocal(out=rinvJ[:], in_=pb[5][:, 0:1])
        nc.vector.tensor_scalar_mul(attnJ[:], emJ[:], rinvJ[:])
        nc.scalar.dma_start(out=outj_e[:], in_=attnJ[:])

    nc.compile()
    return nc


def _get_nc(Tc):
    if _STATE.get("Tc") != Tc:
        _STATE["nc"] = _build_nc(Tc)
        _STATE["Tc"] = Tc
    return _STATE["nc"]


def _make_in_maps(query, key, mask, W, bias):
    query = np.asarray(query, dtype=np.float32)
    key = np.asarray(key, dtype=np.float32)
    mask = np.asarray(mask, dtype=np.float32)
    W = np.asarray(W, dtype=np.float32)
    bias = np.asarray(bias, dtype=np.float32).reshape(-1)

    if COMPACT:
        kept = [np.flatnonzero(mask[b] > 0.5) for b in range(B)]
        ns = np.array([len(k) for k in kept])
        Tc = int(-(-ns.max() // 8) * 8)  # round up to multiple of 8
        if Tc > T:
            Tc = T
    else:
        kept = [np.arange(T) for _ in range(B)]
        ns = np.full(B, T)
        Tc = T
    _STATE["kept"] = kept
    _STATE["ns"] = ns
    _STATE["cur_Tc"] = Tc

    # wt[p, kc, qc, m] = W[kc*128+m, qc*128+p]  (kc-major; shared)
    WT = np.ascontiguousarray(
        np.ascontiguousarray(W.T).reshape(QC, P, KC, P).transpose(1, 2, 0, 3)
    ).astype(np.float16)
    biasb = np.ascontiguousarray(
        np.broadcast_to(bias[:1][None, :], (P, 1)).astype(np.float32)
    )
    CFJ = -(-Tc // JB)
    _STATE["CFJ"] = CFJ
    pj = np.arange(P) // JB
    GRPJ = np.ascontiguousarray((pj[:, None] == pj[None, :]).astype(np.float32))
    IDN = np.ascontiguousarray(np.eye(P, dtype=np.float16))

    in_maps = []
    for i in range(NCORES):
        sh = slice(i * BS, (i + 1) * BS)
        qt = np.ascontiguousarray(
            query[sh].T.reshape(QC, P, BS).transpose(1, 0, 2)
        ).astype(np.float16)
        kk = key[sh]
        if COMPACT:
            kg = np.zeros((BS, Tc, K), dtype=np.float16)
            mb = np.zeros((BS, Tc), dtype=np.float32)
            for bb in range(BS):
                kb = kept[i * BS + bb]
                kg[bb, : len(kb)] = kk[bb, kb]
                mb[bb, : len(kb)] = 1.0
        else:
            kg = kk.astype(np.float16)
            mb = np.ascontiguousarray(mask[sh])
        # keyt[pr, kcp, p, j, h*Tc + t] = kg[2*pr+h, t, (2*kcp+j)*128+p]
        # (only pairs 0..6 are consumed by the PE; pair 7 goes via keyj)
        keyt = np.ascontiguousarray(
            kg[: BS - 2].reshape(PR - 1, 2, Tc, KCP, 2, P).transpose(0, 3, 5, 4, 1, 2)
        )
        # DVE pair (batches 14/15): row layout keyj[h*64+j, c, :] =
        # kg-row j*CFJ+c of that batch, zero past the kept count
        keyj = np.zeros((P, CFJ, K), dtype=np.float16)
        maskj = np.zeros((P, CFJ), dtype=np.float32)
        for h in range(2):
            bb = BS - 2 + h
            n = int(ns[i * BS + bb])
            rows = kg[bb, :n]  # [n, K] fp16
            full = np.zeros((JB * CFJ, K), dtype=np.float16)
            full[:n] = rows[: JB * CFJ]
            keyj[h * JB : (h + 1) * JB] = full.reshape(JB, CFJ, K)
            mj = np.zeros(JB * CFJ, dtype=np.float32)
            mj[:n] = 1.0
            maskj[h * JB : (h + 1) * JB] = mj.reshape(JB, CFJ)
        in_maps.append(
            {
                "qt": qt,
                "wt": WT,
                "keyt": keyt,
                "maskb": mb,
                "biasb": biasb,
                "keyj": keyj,
                "maskj": maskj,
                "grpj": GRPJ,
                "idn": IDN,
            }
        )
    return in_maps


def _run(in_maps, **kwargs):
    from concourse.bass_utils import run_bass_kernel_spmd

    return run_bass_kernel_spmd(
        _get_nc(_STATE["cur_Tc"]), in_maps, core_ids=list(range(NCORES)), **kwargs
    )


def _gather(results):
    out = np.zeros((B, T), dtype=np.float32)
    kept = _STATE["kept"]
    ns = _STATE["ns"]
    CFJ = _STATE["CFJ"]
    for i in range(NCORES):
        rows = np.asarray(results[i]["out"]).reshape(BS - 2, _STATE["cur_Tc"])
        for bb in range(BS - 2):
            b = i * BS + bb
            out[b, kept[b][: ns[b]]] = rows[bb, : ns[b]]
        rowsj = np.asarray(results[i]["outj"]).reshape(2, JB * CFJ)
        for h in range(2):
            b = i * BS + BS - 2 + h
            out[b, kept[b][: ns[b]]] = rowsj[h, : ns[b]]
    return out


def kernel(query, key, mask, W, bias):
    in_maps = _make_in_maps(query, key, mask, W, bias)
    res = _run(in_maps)
    return _gather(res.results)
